# revision 20
# baseline (speedup 1.0000x reference)
"""Trainium2 Bass kernel for nn_DifferentiableAggregation (segment_reduce).

Computes, for batch of 8192 segments over 8388608 sub-images:
    s0[g]  = sum over i with idx_i == g of sub_logits[i, 0]
    s12[g] = sum over i with idx_i == g of (sub_logits[i, 1] + sub_logits[i, 2])
    out[g] = [log(sigmoid(10*(1-s12[g])) + 1e-10),
              log(sigmoid(10*(5-s0[g]))  + 1e-10)]

Strategy: shard the sub-image axis across 8 NeuronCores. Each core does a
local segment-sum via one-hot matmuls accumulating in PSUM (index split as
g = r*64 + q with r = idx>>6 on the 128 PSUM partitions and q = idx&63 in
the free dim), then an AllReduce of the [128, 128] partial and the
sigmoid/log epilogue on the scalar engine.

All one-hot/value panels are built in a bins-major ("transposed") layout
[p, bins, j] so every DVE operand has unit inner stride (fp16 2x perf
mode eligible) and the matmuls run in fp16 (1 cyc/row, overlapped weight
loads). Values are kept fp32-exact via an fp16 hi/lo split; the lo panel
is scaled by 2^11 to stay in fp16 normal range and the final PSUM combine
undoes the scale.
"""

import sys

sys.path.insert(0, "/opt/trn_rl_repo")

import numpy as np

from concourse import bass, bacc, mybir, tile
from concourse.bass_utils import run_bass_kernel_spmd

N_CORES = 8
TOTAL = 8388608
SHARD = TOTAL // N_CORES  # 1048576
BATCH = 8192
P = 128
F32 = mybir.dt.float32
F16 = mybir.dt.float16

K_SHARP = 10.0
EPS = 1e-10
LO_SCALE = 2048.0  # 2^11: keeps lo residuals in fp16 normal range


def build_nc(to_count, ti, s_blk=32):
    """Build + compile the SPMD bass program. Per core handles
    to_count * 128 * ti values."""
    shard = to_count * P * ti
    s_blk = min(s_blk, ti)
    nc = bacc.Bacc(
        "TRN2",
        debug=False,
        target_bir_lowering=False,
        num_devices=N_CORES,
    )
    v_in = nc.dram_tensor("v", [shard * 3], F32, kind="ExternalInput")
    r_in = nc.dram_tensor("ridx", [shard], F16, kind="ExternalInput")
    q_in = nc.dram_tensor("qidx", [shard], F32, kind="ExternalInput")
    io128t_in = nc.dram_tensor("io128t", [P, 128 * s_blk], F16,
                               kind="ExternalInput")
    io64t_in = nc.dram_tensor("io64t", [P, 64 * s_blk], F32,
                              kind="ExternalInput")
    out_part = nc.dram_tensor("part", [P, 128], F32, kind="ExternalOutput")
    out_logits = nc.dram_tensor("logits", [2, BATCH], F32, kind="ExternalOutput")

    with tile.TileContext(nc) as tc:
        _kernel_body(tc, to_count, ti, s_blk, v_in, r_in, q_in, io128t_in,
                     io64t_in, out_part, out_logits)
    nc.compile()
    return nc


def _kernel_body(tc, to_count, ti, S, v_in, r_in, q_in, io128t_in, io64t_in,
                 out_part, out_logits):
    nc = tc.nc
    add = mybir.AluOpType.add
    sub = mybir.AluOpType.subtract
    is_equal = mybir.AluOpType.is_equal
    mult = mybir.AluOpType.mult
    AF = mybir.ActivationFunctionType

    assert ti % S == 0
    nb = ti // S

    v3 = v_in.ap().rearrange("(o p f) -> o p f", p=P, f=ti * 3)
    rv = r_in.ap().rearrange("(o p f) -> o p f", p=P, f=ti)
    qv = q_in.ap().rearrange("(o p f) -> o p f", p=P, f=ti)

    with (
        tc.tile_pool(name="const", bufs=1) as cpool,
        tc.tile_pool(name="data", bufs=2) as dpool,
        tc.tile_pool(name="onehot", bufs=2) as bpool,
        tc.tile_pool(name="mid", bufs=2) as mpool,
        tc.tile_pool(name="psum", bufs=1, space="PSUM") as ppool,
        tc.tile_pool(name="epi", bufs=1) as epool,
        tc.tile_pool(name="dram", bufs=1, space="DRAM") as drampool,
    ):
        # constants: bins-major iota tables, element [p, k*S + j] = k
        io128t = cpool.tile([P, 128 * S], F16)
        nc.sync.dma_start(io128t[:], io128t_in.ap())
        io64t = cpool.tile([P, 64 * S], F32)
        nc.sync.dma_start(io64t[:], io64t_in.ap())
        io128t3 = io128t[:].rearrange("p (k j) -> p k j", j=S)
        io64t3 = io64t[:].rearrange("p (k j) -> p k j", j=S)

        acc = ppool.tile([P, 256], F32)

        for to in range(to_count):
            vt = dpool.tile([P, ti * 3], F32, tag="vt")
            nc.sync.dma_start(vt[:], v3[to])
            rt = dpool.tile([P, ti], F16, tag="rt")
            nc.sync.dma_start(rt[:], rv[to])
            qt = dpool.tile([P, ti], F32, tag="qt")
            nc.sync.dma_start(qt[:], qv[to])

            # hi/lo fp16 value prep (exact to ~2^-22):
            #   chi = fp16(c); clo = fp16((c - fp32(chi)) * 2^11)
            vt3 = vt[:].rearrange("p (t c) -> p t c", c=3)
            c12 = dpool.tile([P, ti], F32, tag="c12")
            nc.vector.tensor_tensor(c12[:], vt3[:, :, 1], vt3[:, :, 2], add)
            chi0 = dpool.tile([P, ti], F16, tag="chi0")
            nc.vector.tensor_copy(chi0[:], vt3[:, :, 0])
            chi12 = dpool.tile([P, ti], F16, tag="chi12")
            nc.vector.tensor_copy(chi12[:], c12[:])
            back0 = dpool.tile([P, ti], F32, tag="back0")
            nc.vector.tensor_scalar(back0[:], chi0[:], LO_SCALE, None, mult)
            back12 = dpool.tile([P, ti], F32, tag="back12")
            nc.vector.tensor_scalar(back12[:], chi12[:], LO_SCALE, None, mult)
            clo0 = dpool.tile([P, ti], F16, tag="clo0")
            nc.vector.scalar_tensor_tensor(clo0[:], vt3[:, :, 0], LO_SCALE,
                                           back0[:], mult, sub)
            clo12 = dpool.tile([P, ti], F16, tag="clo12")
            nc.vector.scalar_tensor_tensor(clo12[:], c12[:], LO_SCALE,
                                           back12[:], mult, sub)

            for b in range(nb):
                sl = slice(b * S, (b + 1) * S)

                # r one-hot, bins-major: B_T[p, k, j] = (r[p, j] == k), fp16
                B_T = bpool.tile([P, 128 * S], F16, tag="B")
                B3 = B_T[:].rearrange("p (k j) -> p k j", j=S)
                rbt = (
                    rt[:, sl]
                    .rearrange("p (o j) -> p o j", o=1)
                    .to_broadcast([P, 128, S])
                )
                nc.vector.tensor_tensor(B3, rbt, io128t3, is_equal)

                # q difference on gpsimd (contiguous streams), fp32
                D_T = mpool.tile([P, 64 * S], F32, tag="D")
                D3 = D_T[:].rearrange("p (k j) -> p k j", j=S)
                qbt = (
                    qt[:, sl]
                    .rearrange("p (o j) -> p o j", o=1)
                    .to_broadcast([P, 64, S])
                )
                nc.gpsimd.tensor_tensor(D3, qbt, io64t3, sub)

                # q one-hot on the scalar engine: relu(1 - |D|), fp16 out
                AB_T = mpool.tile([P, 64 * S], F32, tag="AB")
                nc.scalar.activation(AB_T[:], D_T[:], AF.Abs, bias=0.0,
                                     scale=1.0)
                OHQ_T = mpool.tile([P, 64 * S], F16, tag="OHQ")
                OHQ3 = OHQ_T[:].rearrange("p (k j) -> p k j", j=S)
                nc.scalar.activation(OHQ_T[:], AB_T[:], AF.Relu, bias=1.0,
                                     scale=-1.0)

                # value panels: [hi0 | lo0 | hi12 | lo12], all fp16,
                # bins-major, unit inner strides
                VQ_T = bpool.tile([P, 4 * 64 * S], F16, tag="VQ")
                VQ4 = VQ_T[:].rearrange("p (c k j) -> p c k j", c=4, j=S)
                for ci, csrc in enumerate((chi0, clo0, chi12, clo12)):
                    cbt = (
                        csrc[:, sl]
                        .rearrange("p (o j) -> p o j", o=1)
                        .to_broadcast([P, 64, S])
                    )
                    nc.vector.tensor_tensor(VQ4[:, ci], cbt, OHQ3, mult)

                VQr = VQ_T[:].rearrange("p (c k j) -> p j c k", c=4, j=S)
                for j in range(S):
                    first = to == 0 and b == 0 and j == 0
                    last = to == to_count - 1 and b == nb - 1 and j == S - 1
                    nc.tensor.matmul(
                        acc[:],
                        lhsT=B3[:, :, j],
                        rhs=VQr[:, j],
                        start=first,
                        stop=last,
                    )

        # Combine hi + lo/2^11 panels, publish partial, AllReduce, epilogue
        a_sb = epool.tile([P, 256], F32)
        nc.vector.tensor_copy(a_sb[:], acc[:])
        s_sb = epool.tile([P, 128], F32)
        nc.vector.scalar_tensor_tensor(s_sb[:, 0:64], a_sb[:, 64:128],
                                       1.0 / LO_SCALE, a_sb[:, 0:64], mult, add)
        nc.vector.scalar_tensor_tensor(s_sb[:, 64:128], a_sb[:, 192:256],
                                       1.0 / LO_SCALE, a_sb[:, 128:192], mult,
                                       add)
        nc.sync.dma_start(out_part.ap(), s_sb[:])

        din = drampool.tile([P, 128], F32)
        dout = drampool.tile([P, 128], F32)
        nc.gpsimd.dma_start(din[:], s_sb[:])
        nc.gpsimd.collective_compute(
            "AllReduce",
            add,
            replica_groups=[list(range(N_CORES))],
            ins=[din.opt()],
            outs=[dout.opt()],
        )
        sf = epool.tile([P, 128], F32)
        nc.gpsimd.dma_start(sf[:], dout[:])

        # Epilogue: out_c = log(sigmoid(z) + eps), z = -10*s + bias_c.
        # sigmoid computed exactly as 1/(1 + exp(-z)) (ACT exp table +
        # accurate DVE reciprocal); -z clamped at 88 to avoid exp overflow.
        beps = epool.tile([P, 1], F32)
        nc.vector.memset(beps[:], EPS)

        def logsig(out_ap, s_ap, zbias):
            mz = epool.tile([P, 64], F32, tag="mz")
            nc.vector.tensor_scalar(mz[:], s_ap, K_SHARP, -zbias, mult, add)
            nc.vector.tensor_scalar(mz[:], mz[:], 88.0, None,
                                    mybir.AluOpType.min)
            w = epool.tile([P, 64], F32, tag="w")
            nc.scalar.activation(w[:], mz[:], AF.Exp, bias=0.0, scale=1.0)
            nc.vector.tensor_scalar(w[:], w[:], 1.0, None, add)
            r_ = epool.tile([P, 64], F32, tag="r_")
            nc.vector.reciprocal(r_[:], w[:])
            nc.scalar.activation(out_ap, r_[:], AF.Ln, bias=beps[:], scale=1.0)

        o1 = epool.tile([P, 64], F32)
        logsig(o1[:], sf[:, 64:128], K_SHARP)
        o0 = epool.tile([P, 64], F32)
        logsig(o0[:], sf[:, 0:64], 5.0 * K_SHARP)

        ol = out_logits.ap().rearrange("w (p t) -> w p t", p=P, t=BATCH // P)
        nc.sync.dma_start(ol[0], o1[:])
        nc.sync.dma_start(ol[1], o0[:])


_NC_CACHE = {}


def _get_nc(to_count, ti, s_blk=32):
    key = (to_count, ti, s_blk)
    if key not in _NC_CACHE:
        _NC_CACHE[key] = build_nc(to_count, ti, s_blk)
    return _NC_CACHE[key]


def make_in_maps(sub_logits, original_indices, to_count, ti, s_blk=32):
    shard = to_count * P * ti
    s_blk = min(s_blk, ti)
    idx = np.asarray(original_indices).astype(np.int32)
    v = np.ascontiguousarray(np.asarray(sub_logits, dtype=np.float32)).reshape(-1)
    r_f = (idx >> 6).astype(np.float16)
    q_f = (idx & 63).astype(np.float32)
    io128t = np.ascontiguousarray(
        np.broadcast_to(
            np.repeat(np.arange(128, dtype=np.float16), s_blk)[None, :],
            (P, 128 * s_blk),
        )
    )
    io64t = np.ascontiguousarray(
        np.broadcast_to(
            np.repeat(np.arange(64, dtype=np.float32), s_blk)[None, :],
            (P, 64 * s_blk),
        )
    )
    vs = v.reshape(N_CORES, shard * 3)
    rs = r_f.reshape(N_CORES, shard)
    qs = q_f.reshape(N_CORES, shard)
    return [
        {
            "v": vs[c],
            "ridx": rs[c],
            "qidx": qs[c],
            "io128t": io128t,
            "io64t": io64t,
        }
        for c in range(N_CORES)
    ]


def kernel(sub_logits, original_indices, batch_size=None, _trace=False):
    to_count, ti, s_blk = 16, 512, 32
    nc = _get_nc(to_count, ti, s_blk)
    in_maps = make_in_maps(sub_logits, original_indices, to_count, ti, s_blk)
    res = run_bass_kernel_spmd(
        nc, in_maps, core_ids=list(range(N_CORES)), trace=_trace
    )
    logits = res.results[0]["logits"]
    out = np.stack([logits[0], logits[1]], axis=1).astype(np.float32)
    if _trace:
        kernel._last_results = res
    return out
